# revision 2
# baseline (speedup 1.0000x reference)
"""Distributed multi-head attention kernel for 8 TRN2 NeuronCores.

Problem: B=2, S=2048, D=1024, H=16 heads (hd=64).
  qkv = x @ w_qkv.T ; attention per head ; out = attn @ w_out.T

Sharding (no hardware collectives needed):
  core c -> batch b = c // 4, head-group g = c % 4 (heads 4g..4g+3).
  Each core computes a *partial* output projection (its 256 attn channels
  against the full w_out columns); the host sums the 4 partials per batch.

Per-core layout trick: the host pre-transposes x and the weight shards so
every matmul operand arrives with its contraction dim on partitions --
zero on-chip transposes. Inputs are staged host-side as [128, ktile, cols]
so each full tensor (or column chunk) loads with ONE wide DMA -- the DMA
path has ~625ns issue + ~2.2us pipe latency per instruction, so few big
DMAs beat many small ones.

Emission is a step machine: the attention inner loop (scores -> exp ->
PV per 128-key tile) is ACT-paced (exp of a [128,1024] tile costs
~1040ns vs ~850ns of PE work per step), so projection / V / output-
projection matmuls are interleaved INTO the kt-step stream as PE
fillers, placed between the next step's scores and the current step's
PV. PE then never idles waiting on exp.
"""

import sys

sys.path.insert(0, "/opt/trn_rl_repo")

import numpy as np
import ml_dtypes

import concourse.bass as bass  # noqa: F401
import concourse.mybir as mybir
import concourse.tile as tile
from concourse import bacc
from concourse.bass_utils import run_bass_kernel_spmd

B, S, D, H = 2, 2048, 1024, 16
HL = 4          # heads per core
HD = 64         # head dim
EL = HL * HD    # local attn channels (256)
N_CORES = 8

f32 = mybir.dt.float32
f32r = mybir.dt.float32r
bf16 = mybir.dt.bfloat16
AF = mybir.ActivationFunctionType

SCALE = 1.0 / (HD ** 0.5)

_CACHE = {}


def build_nc(n_reps=1):
    nc = bacc.Bacc("TRN2", target_bir_lowering=False, debug=False,
                   num_devices=N_CORES)
    KD = 8            # d contraction tiles
    NS = S // 128     # 16 k-tiles of 128

    # host-staged layouts: [128 partitions, tile-index, columns]
    xt = nc.dram_tensor("xt", [128, KD, S], bf16, kind="ExternalInput")
    wqk = nc.dram_tensor("wqk", [128, KD, 2 * EL], bf16, kind="ExternalInput")
    wv = nc.dram_tensor("wv", [128, KD, EL], bf16, kind="ExternalInput")
    wo = nc.dram_tensor("wo", [128, 2, D], bf16, kind="ExternalInput")
    out = nc.dram_tensor("out", [128, 8, S], bf16, kind="ExternalOutput")

    with tile.TileContext(nc) as tc:
        with tc.tile_pool(name="const", bufs=1) as const, \
             tc.tile_pool(name="ps", bufs=1, space="PSUM") as ps, \
             tc.tile_pool(name="expp", bufs=4) as expp, \
             tc.tile_pool(name="smalls", bufs=6) as smalls, \
             tc.tile_pool(name="outp", bufs=4) as outp:
          for _rep in range(n_reps):
            xt_sb = const.tile([128, KD, S], bf16)
            wqk_sb = const.tile([128, KD, 2 * EL], bf16)
            wv_sb = const.tile([128, KD, EL], bf16)
            wo_sb = const.tile([128, 2, D], bf16)
            qkT = const.tile([128, 4, S], bf16)
            vhat = const.tile([128, NS, HL, 128], bf16)
            attnT = const.tile([128, 2, S], bf16)

            # ---- input DMAs: one wide DMA per tensor/chunk, ordered by
            # first use ----
            nc.sync.dma_start(out=wqk_sb[:, 0:4, 256:512], in_=wqk[:, 0:4, 256:512])
            for j2 in range(4):
                nc.sync.dma_start(out=xt_sb[:, 2 * j2:2 * j2 + 2, 0:512],
                                  in_=xt[:, 2 * j2:2 * j2 + 2, 0:512])
            nc.sync.dma_start(out=wqk_sb[:, 4:8, 256:512], in_=wqk[:, 4:8, 256:512])
            nc.sync.dma_start(out=wqk_sb[:, :, 0:256], in_=wqk[:, :, 0:256])
            nc.sync.dma_start(out=wv_sb, in_=wv[:, :, :])
            for n in range(1, 4):
                nc.sync.dma_start(out=xt_sb[:, :, n * 512:(n + 1) * 512],
                                  in_=xt[:, :, n * 512:(n + 1) * 512])
            nc.sync.dma_start(out=wo_sb, in_=wo[:, :, :])

            nc.vector.memset(vhat[:, :, :, HD + 1:], 0.0)
            for h in range(HL):
                nc.vector.memset(vhat[:, :, h, HD:HD + 1], 1.0)

            # ---- filler units (atomic small matmul groups) ----
            def unit_qk(m, n):
                p = ps.tile([128, 512], f32, tag="p", bufs=2)
                for k in range(KD):
                    nc.tensor.matmul(
                        p,
                        wqk_sb[:, k, m * 128:(m + 1) * 128],
                        xt_sb[:, k, n * 512:(n + 1) * 512],
                        start=(k == 0), stop=(k == KD - 1))
                nc.vector.tensor_copy(qkT[:, m, n * 512:(n + 1) * 512], p)

            def unit_v(mv):
                p = ps.tile([128, 512], f32, tag="p", bufs=2)
                for k in range(KD):
                    nc.tensor.matmul(
                        p[:, 0:EL],
                        xt_sb[:, k, mv * 128:(mv + 1) * 128],
                        wv_sb[:, k, :],
                        start=(k == 0), stop=(k == KD - 1))
                # single strided copy into the [HL, HD+1] vhat layout
                nc.vector.tensor_copy(vhat[:, mv, :, 0:HD], p[:, 0:EL])

            def unit_op2(q, mp):
                # output projection for column-slab q, row tiles 2mp,2mp+1
                qs = slice(q * 512, (q + 1) * 512)
                ot = outp.tile([128, 2, 512], bf16)
                for i in range(2):
                    m = 2 * mp + i
                    p = ps.tile([128, 512], f32, tag="p", bufs=2)
                    nc.tensor.matmul(
                        p, wo_sb[:, 0, m * 128:(m + 1) * 128],
                        attnT[:, 0, qs], start=True, stop=False)
                    nc.tensor.matmul(
                        p, wo_sb[:, 1, m * 128:(m + 1) * 128],
                        attnT[:, 1, qs], start=False, stop=True)
                    nc.vector.tensor_copy(ot[:, i, :], p)
                nc.sync.dma_start(out=out[:, 2 * mp:2 * mp + 2, qs], in_=ot)

            # ---- attention slab as a generator: yields once per kt step
            # at the point where PE fillers should be injected (between
            # the next step's scores and this step's PV). ----
            def gen_slab(q, hp):
                qs = slice(q * 512, (q + 1) * 512)
                poA = ps.tile([128, 512], f32, tag="poA", bufs=1)
                poB = ps.tile([128, 512], f32, tag="poB", bufs=1)

                def scores(kt):
                    ks = slice(kt * 128, (kt + 1) * 128)
                    sp = ps.tile([128, 1024], f32, tag="sp", bufs=2)
                    nc.tensor.matmul(
                        sp[:, 0:512],
                        qkT[0:64, 2 + hp, ks], qkT[0:64, hp, qs],
                        start=True, stop=True, tile_position=(0, 0))
                    nc.tensor.matmul(
                        sp[:, 512:1024],
                        qkT[64:128, 2 + hp, ks], qkT[64:128, hp, qs],
                        start=True, stop=True, tile_position=(64, 0))
                    return sp

                sp_next = scores(0)
                for kt in range(NS):
                    sp_cur = sp_next
                    if kt + 1 < NS:
                        sp_next = scores(kt + 1)
                    yield kt  # <- filler injection point
                    et = expp.tile([128, 1024], bf16)
                    nc.scalar.activation(et, sp_cur, AF.Exp, scale=SCALE)
                    nc.tensor.matmul(
                        poA, vhat[:, kt, 2 * hp, :], et[:, 0:512],
                        start=(kt == 0), stop=(kt == NS - 1))
                    nc.tensor.matmul(
                        poB, vhat[:, kt, 2 * hp + 1, :], et[:, 512:1024],
                        start=(kt == 0), stop=(kt == NS - 1))

                # finalize: stage po to SBUF, normalize into attnT
                stA = smalls.tile([HD + 1, 512], f32)
                stB = smalls.tile([HD + 1, 512], f32)
                nc.vector.tensor_copy(stA, poA[0:HD + 1, :])
                nc.vector.tensor_copy(stB, poB[0:HD + 1, :])
                recA = smalls.tile([1, 512], f32)
                recB = smalls.tile([1, 512], f32)
                nc.vector.reciprocal(recA, stA[64:65, :])
                nc.vector.reciprocal(recB, stB[64:65, :])
                bcA = smalls.tile([64, 512], f32)
                bcB = smalls.tile([64, 512], f32)
                nc.gpsimd.partition_broadcast(bcA, recA)
                nc.gpsimd.partition_broadcast(bcB, recB)
                nc.vector.tensor_mul(attnT[0:64, hp, qs], stA[0:64, :], bcA)
                nc.vector.tensor_mul(attnT[64:128, hp, qs], stB[0:64, :], bcB)

            # ---- per-slab filler schedules: {step: [unit thunks]} ----
            QK, V, OP = unit_qk, unit_v, unit_op2
            u = lambda f, *a: (lambda: f(*a))
            slab_fill = {
                (0, 0): {
                    0: [u(V, 1)], 1: [u(V, 2), u(QK, 2, 1)], 2: [u(V, 3)],
                    3: [u(V, 4)], 4: [u(V, 5), u(QK, 2, 2)], 5: [u(V, 6)],
                    6: [u(V, 7)], 7: [u(V, 8)], 8: [u(V, 9), u(QK, 2, 3)],
                    9: [u(V, 10)], 10: [u(V, 11), u(QK, 3, 0)],
                    11: [u(V, 12), u(QK, 3, 1)], 12: [u(V, 13), u(QK, 3, 2)],
                    13: [u(V, 14), u(QK, 3, 3)], 14: [u(V, 15), u(QK, 1, 0)],
                },
                # OP(q) units go one full slab AFTER finalize(q,1) so they
                # never chain through the fresh recip->bcast->mul (the DVE
                # reciprocal is ~2us on HW); the dependency-free QK units
                # fill the slab right after each finalize instead.
                (0, 1): {2: [u(QK, 0, 1)], 8: [u(QK, 1, 1)]},
                (1, 0): {2: [u(QK, 0, 2)], 8: [u(QK, 1, 2)]},
                (1, 1): {1 + 4 * i: [u(OP, 0, i)] for i in range(4)},
                (2, 0): {2: [u(QK, 0, 3)], 8: [u(QK, 1, 3)]},
                (2, 1): {1 + 4 * i: [u(OP, 1, i)] for i in range(4)},
                (3, 0): {8: [u(OP, 2, 0)], 12: [u(OP, 2, 1)]},
                (3, 1): {2: [u(OP, 2, 2)], 8: [u(OP, 2, 3)]},
            }

            # ---- main emission ----
            unit_qk(2, 0)
            unit_qk(0, 0)
            unit_v(0)
            for (q, hp), fills in slab_fill.items():
                g = gen_slab(q, hp)
                for kt in g:
                    for thunk in fills.get(kt, ()):
                        thunk()
            for mp in range(4):
                unit_op2(3, mp)

    nc.finalize()
    return nc


def _stage3(a, ktiles):
    """[R, C] row-major -> [128, R//128, C] (partition-major staging)."""
    R, C = a.shape
    assert R == 128 * ktiles
    return np.ascontiguousarray(a.reshape(ktiles, 128, C).transpose(1, 0, 2))


def make_in_maps(x, w_qkv, w_out):
    x = np.asarray(x, dtype=np.float32)
    w_qkv = np.asarray(w_qkv, dtype=np.float32)
    w_out = np.asarray(w_out, dtype=np.float32)
    in_maps = []
    for c in range(N_CORES):
        b, g = divmod(c, 4)
        r0 = g * EL
        wq = w_qkv[r0:r0 + EL]
        wk = w_qkv[D + r0:D + r0 + EL]
        wv_ = w_qkv[2 * D + r0:2 * D + r0 + EL]
        in_maps.append({
            "xt": _stage3(x[b].T, 8).astype(ml_dtypes.bfloat16),
            "wqk": _stage3(np.concatenate([wq, wk], axis=0).T, 8).astype(ml_dtypes.bfloat16),
            "wv": _stage3(wv_.T, 8).astype(ml_dtypes.bfloat16),
            "wo": _stage3(w_out[:, r0:r0 + EL].T, 2).astype(ml_dtypes.bfloat16),
        })
    return in_maps


def kernel(x, w_qkv, w_out):
    if "nc" not in _CACHE:
        _CACHE["nc"] = build_nc()
    nc = _CACHE["nc"]
    in_maps = make_in_maps(x, w_qkv, w_out)
    res = run_bass_kernel_spmd(nc, in_maps, core_ids=list(range(N_CORES)))
    final = np.empty((B, S, D), dtype=np.float32)
    for b in range(B):
        acc = res.results[4 * b]["out"].astype(np.float32)
        for g in range(1, 4):
            acc += res.results[4 * b + g]["out"].astype(np.float32)
        # [128, 8, S] -> [D, S] -> [S, D]
        final[b] = acc.transpose(1, 0, 2).reshape(D, S).T
    return final
